# revision 23
# baseline (speedup 1.0000x reference)
"""Trainium2 Bass kernel for nn_MultiHeadAttention_37538014167348.

The reference einsum is 'bhqk,bhvd->bhqd' (k and v are independent), so the
attention output factorizes into (sum_k softmax_weights) * (sum_v V). Softmax
rows sum to exactly 1 (also true for the complex softmax), hence:

    out[b, q, :] = (sum_s x[b, s, :]) @ Wv + S * bv     (independent of q)

Q/K/mask/softmax drop out entirely. The kernel computes the row-sum of x and a
complex [1,768]x[768,768] matvec; the host broadcasts the resulting row over
the 1024 sequence positions.

Sharding over 8 cores: (batch b in 0..3) x (contraction/feature half). Core
(b, j) reads x[b, :, j*384:(j+1)*384] (all 1024 rows, half the features,
3.15MB) and Wv[j*384:(j+1)*384, :] (half the weight rows, full 768 output
columns, 1.18MB bf16), and produces the partial matvec y_bj = u_bj @ Wv[half].
The host sums the two partials per batch and adds S*bv: no cross-core
communication, and per-core DMA drops from 7.44MB (previous version) to
4.33MB, which is what bounds the kernel (HBM ~358GB/s).

Pipeline (per core): x arrives as two column slabs: cols [0:512) as 8
row-subs of 128 rows (2KB DMA elements, on the two HWDGE queues), and cols
[512:768) as 4 pair-packed row-subs (1KB elements, on the gpsimd queue with
the weights). Stage 1 reduces rows with a stationary ones [128,2] f32r matmul
(free size 512 keeps the PE at 1 cycle/row) into psum uA [2,512] and uB
[2,512] (= even-row | odd-row partials, folded during the deinterleave adds).
Per 128-complex-feature chunk: DVE deinterleaves re/im psum columns into
rows, one PE transpose yields u columns [a|b], DVE packs bf16 [a,b] and
[-b,a] pairs, and 4 bf16 matmuls per chunk accumulate y into psum
[2,512]+[2,256] against the merged weight planes [C_cc | D_cc]. A dummy
matmul chain spans the DMA lead-in so the PE clock is ramped when real work
starts; constants are built on-chip (tiny-packet DMAs poison a queue).
Output is bf16 [2,768] (cast split across DVE and Act engines).
"""

import os
import sys

import numpy as np

for _p in ("/opt/trn_rl_repo", "/root/.axon_site/_ro/trn_rl_repo"):
    if os.path.isdir(_p) and _p not in sys.path:
        sys.path.append(_p)

import ml_dtypes

from concourse import bacc, mybir
from concourse.tile import TileContext
from concourse.bass_utils import run_bass_kernel_spmd

B, S, H = 4, 1024, 768
HALF = H // 2           # complex features per core (384) = contraction half
NCORES = 8
P = 128                 # SBUF partitions
CC = HALF // P          # 3 contraction chunks of 128 complex features
F32 = mybir.dt.float32
F32R = mybir.dt.float32r
BF16 = mybir.dt.bfloat16

_NC = None
LAST_RESULTS = None     # stashed BassKernelResults for profiling in test.py


def _build():
    nc = bacc.Bacc(None, target_bir_lowering=False)

    # per-core x half: f32 view of x[b, :, j*384:(j+1)*384], contiguous
    x = nc.dram_tensor("x", [S, 2 * HALF], F32R, kind="ExternalInput")
    # merged per-chunk weight planes [C_cc | D_cc]: w{cc}[p, 0:768] =
    # bf16(Re(Wv)[joff+cc*128+p, :]), [768:1536] = Im. 3KB rows -> best DMA
    # packet size.
    ws = [nc.dram_tensor(f"w{cc}", [P, 2 * H], BF16, kind="ExternalInput")
          for cc in range(CC)]
    idf = nc.dram_tensor("idf", [2, 512], F32, kind="ExternalInput")
    o = nc.dram_tensor("o", [2, H], BF16, kind="ExternalOutput")

    with TileContext(nc) as tc:
        with tc.tile_pool(name="sbuf", bufs=1) as pool, \
             tc.tile_pool(name="psum", bufs=1, space="PSUM") as psum:

            onesP = pool.tile([P, 2], F32R)
            ones_f = pool.tile([P, 2], F32)
            id2f = pool.tile([2, 512], F32)
            dummy_f = pool.tile([P, 512], F32)
            dummy_mv = pool.tile([P, 512], F32R)
            w_sb = [pool.tile([P, 2 * H], BF16, name=f"wsb{cc}") for cc in range(CC)]

            # slab01: x f32 cols [0:512) as 8 row-subs of 128 rows (2KB DMA
            # elements); group g holds rows 128g+p -> 8 free-512 matmuls.
            t01 = pool.tile([P, 8, 512], F32R)
            # slab2: x f32 cols [512:768) as 4 row-subs of 256 rows (1KB DMA
            # elements); partition p of sub s holds rows 256s+2p (group 2s)
            # and 256s+2p+1 (group 2s+1): one free-512 matmul per sub into
            # uB [2,512] = [even-row feats | odd-row feats], folded during
            # the deinterleave adds.
            t2 = pool.tile([P, 8, 256], F32R)

            # ---- on-chip constants: NEVER DMA [128, small] tiles (tiny
            # packets poison the queue). ones and the transpose identity are
            # built with memsets; id2's diagonal writes use partition-offset
            # memsets.
            nc.gpsimd.memset(ones_f[:], 1.0)
            nc.gpsimd.memset(dummy_f[:], 0.0)
            nc.vector.tensor_copy(onesP[:], ones_f[:])
            nc.vector.tensor_copy(dummy_mv[:], dummy_f[:])
            # identity rides as [2, 512] f32 = two 2KB packets (tiny-packet
            # transfers poison a queue); only [0:2, 0:2] is used.
            nc.gpsimd.dma_start(out=id2f[:], in_=idf[:, :])
            id2 = id2f[0:2, 0:2]

            # ---- DMA triggers. Queues sustain ~118GB/s each (aggregate
            # HBM-bound ~350GB/s); scalar/sync HWDGE start ~8.8us, gpsimd
            # swdge ~10.5us. The a-slab (stage-1 critical) goes first
            # everywhere; weights mid-stream; the last bytes are z2/w2 whose
            # dependent chains are shortest.
            def asub(g, eng):
                eng.dma_start(out=t01[:, g, :],
                              in_=x[P * g:P * (g + 1), 0:512])

            def zsub(s, eng):
                eng.dma_start(out=t2[:, 2 * s:2 * s + 2, :],
                              in_=x[256 * s:256 * (s + 1), 512:768])

            asub(0, nc.scalar)
            asub(1, nc.sync)
            asub(2, nc.gpsimd)
            asub(3, nc.scalar)
            asub(4, nc.sync)
            asub(5, nc.gpsimd)
            asub(6, nc.scalar)
            asub(7, nc.sync)
            nc.scalar.dma_start(out=w_sb[0][:], in_=ws[0][:, :])
            nc.sync.dma_start(out=w_sb[1][:], in_=ws[1][:, :])
            nc.gpsimd.dma_start(out=t2[:, 0:4, :], in_=x[0:512, 512:768])
            zsub(2, nc.scalar)
            zsub(3, nc.sync)
            nc.gpsimd.dma_start(out=w_sb[2][:], in_=ws[2][:, :])

            # ---- psum: uA (feats 0-511), uB (feats 512-767 pair-folded),
            # transposes, stage-2 accumulators, and the warm-up scratch.
            uA = psum.tile([2, 512], F32)
            uB = psum.tile([2, 512], F32)
            u_row = [pool.tile([2, P], F32, name=f"urow{cc}") for cc in range(CC)]
            tp = [psum.tile([P, 2], F32, name=f"tp{cc}") for cc in range(CC)]
            u_ab = [pool.tile([P, 2], BF16, name=f"uab{cc}") for cc in range(CC)]
            u_bna = [pool.tile([P, 2], BF16, name=f"ubna{cc}") for cc in range(CC)]
            oA = psum.tile([2, 512], F32)
            oB = psum.tile([2, 256], F32)
            ub_sb = pool.tile([2, 256], F32)
            scratch = psum.tile([2, 512], F32)

            def deint(cc):
                # u_row[cc]: row0 = a (Re), row1 = b (Im), feats cc*128..+127
                if cc < 2:
                    v = uA.rearrange("q (f two) -> q two f", two=2)
                    base = cc * P
                    nc.vector.tensor_copy(u_row[cc][0:2, :],
                                          v[0:2, 1, base:base + P])
                    nc.vector.tensor_copy(u_row[cc][0:1, :],
                                          v[0:1, 0, base:base + P])
                else:
                    # fold even-row + odd-row halves while deinterleaving;
                    # tensor_add reads at most one PSUM input, so bounce the
                    # even half through SBUF.
                    nc.vector.tensor_copy(ub_sb[:], uB[:, 0:256])
                    vp = uB.rearrange("q (g f two) -> q g two f", g=2, two=2)
                    vs = ub_sb.rearrange("q (f two) -> q two f", two=2)
                    nc.vector.tensor_add(u_row[cc][0:2, :],
                                         vp[0:2, 1, 1, :], vs[0:2, 1, :])
                    nc.vector.tensor_add(u_row[cc][0:1, :],
                                         vp[0:1, 1, 0, :], vs[0:1, 0, :])

            def pack(cc):
                nc.tensor.transpose(tp[cc][:, :], u_row[cc][0:2, :], id2)
                nc.vector.tensor_copy(u_ab[cc][:], tp[cc][:])
                nc.vector.tensor_scalar_mul(u_bna[cc][:, 0:1], tp[cc][:, 1:2], -1.0)
                nc.vector.tensor_copy(u_bna[cc][:, 1:2], tp[cc][:, 0:1])

            def stage2(cc, first, last):
                if not last:
                    nc.tensor.matmul(oA[:, :], u_ab[cc][:, :],
                                     w_sb[cc][:, 0:512], start=first, stop=False)
                    nc.tensor.matmul(oB[:, :], u_ab[cc][:, :],
                                     w_sb[cc][:, 512:768], start=first, stop=False)
                    nc.tensor.matmul(oA[:, :], u_bna[cc][:, :],
                                     w_sb[cc][:, 768:1280], start=False, stop=False)
                    nc.tensor.matmul(oB[:, :], u_bna[cc][:, :],
                                     w_sb[cc][:, 1280:1536], start=False, stop=False)
                else:
                    # close oA first: its bf16 cast (DVE, ~0.7us on only 2
                    # partitions) overlaps the remaining oB matmuls
                    nc.tensor.matmul(oA[:, :], u_ab[cc][:, :],
                                     w_sb[cc][:, 0:512], start=False, stop=False)
                    nc.tensor.matmul(oA[:, :], u_bna[cc][:, :],
                                     w_sb[cc][:, 768:1280], start=False, stop=True)
                    nc.tensor.matmul(oB[:, :], u_ab[cc][:, :],
                                     w_sb[cc][:, 512:768], start=False, stop=False)
                    nc.tensor.matmul(oB[:, :], u_bna[cc][:, :],
                                     w_sb[cc][:, 1280:1536], start=False, stop=True)

            # ---- PE warm-up: the clock ramps only under ~3us of continuous
            # execution; idle resets it. Wide dummy matmuls span the DMA
            # lead-in so the real chain starts warm.
            NWARM = 4
            for i in range(NWARM):
                nc.tensor.matmul(scratch[:, :], onesP[:, 0:2], dummy_mv[:, :],
                                 start=(i == 0), stop=(i == NWARM - 1))

            def dummy(n):
                # single-shot scratch matmuls: keep the PE clock ramped
                # through known DMA-stall windows (idle resets the ramp and
                # drops subsequent matmuls to ~1.6x cycle time)
                for _ in range(n):
                    nc.tensor.matmul(scratch[:, :], onesP[:, 0:2],
                                     dummy_mv[:, :], start=True, stop=True)

            # accumulation order is free: gpsimd's subs (a2, a5) arrive
            # latest, so they close the chain instead of head-of-line
            # blocking matmuls whose data is already resident.
            ua_order = (0, 1, 3, 4, 6, 7, 2, 5)
            for i, g in enumerate(ua_order):
                nc.tensor.matmul(uA[:, :], onesP[:, 0:2], t01[:, g, :],
                                 start=(i == 0), stop=(i == 7))
                # one dummy between early-chain matmuls: fills DMA-pacing
                # stalls without delaying a ready matmul by more than one
                # dummy (~0.4us)
                if i in (0, 1, 2, 3) or g == 2:
                    dummy(1)
                if g == 2:
                    dummy(1)
            dummy(1)
            deint(0)
            pack(0)
            deint(1)
            pack(1)
            # uB in expected arrival order (z0 gpsimd, z1 scalar, z2 sync,
            # z3 gpsimd)
            for i, s in enumerate(range(4)):
                nc.tensor.matmul(uB[:, :], onesP[:, 0:2],
                                 t2[:, 2 * s:2 * s + 2, :],
                                 start=(i == 0), stop=(i == 3))
            deint(2)
            pack(2)
            stage2(0, True, False)
            stage2(1, False, False)
            stage2(2, False, True)

            o_sb = pool.tile([2, H], BF16)
            nc.vector.tensor_copy(o_sb[:, 0:512], oA[:])
            nc.scalar.mul(o_sb[:, 512:768], oB[:], 1.0)
            nc.sync.dma_start(out=o[:, :], in_=o_sb[:])

    nc.finalize()
    return nc


def _get_nc():
    global _NC
    if _NC is None:
        _NC = _build()
    return _NC


def _pack_w(Wv, j):
    # merged per-chunk planes [128, 2H]: cols 0:768 = Re rows, 768: = Im rows
    out = []
    for cc in range(CC):
        rows = slice(j * HALF + cc * P, j * HALF + (cc + 1) * P)
        wq = np.empty((P, 2 * H), dtype=ml_dtypes.bfloat16)
        wq[:, 0:H] = Wv.real[rows, :].astype(ml_dtypes.bfloat16)
        wq[:, H:2 * H] = Wv.imag[rows, :].astype(ml_dtypes.bfloat16)
        out.append(np.ascontiguousarray(wq))
    return out


def make_in_maps(x, Wv, bv):
    xf = np.ascontiguousarray(x).view(np.float32).reshape(B, S, 2 * H)
    Wv = np.ascontiguousarray(Wv)
    idv = np.zeros((2, 512), dtype=np.float32)
    idv[0, 0] = 1.0
    idv[1, 1] = 1.0
    wmaps = [_pack_w(Wv, j) for j in range(2)]
    in_maps = []
    for core in range(NCORES):
        b, j = divmod(core, 2)
        xc = np.ascontiguousarray(xf[b][:, j * 2 * HALF:(j + 1) * 2 * HALF])
        im = {"x": xc, "idf": idv}
        for cc in range(CC):
            im[f"w{cc}"] = wmaps[j][cc]
        in_maps.append(im)
    return in_maps


def kernel(x, Wq, bq, Wk, bk, Wv, bv, mask, trace=False):
    global LAST_RESULTS
    x = np.asarray(x)
    Wv = np.asarray(Wv)
    bv = np.asarray(bv)
    in_maps = make_in_maps(x, Wv, bv)
    res = run_bass_kernel_spmd(_get_nc(), in_maps, core_ids=list(range(NCORES)),
                               trace=trace)
    LAST_RESULTS = res
    sbv = (np.complex64(S) * bv).astype(np.complex64)
    row = np.empty((B, H), dtype=np.complex64)
    for b in range(B):
        o0 = res.results[2 * b]["o"].astype(np.float32)      # [2,768] bf16
        o1 = res.results[2 * b + 1]["o"].astype(np.float32)
        row[b] = (o0[0] + o1[0]) + 1j * (o0[1] + o1[1])
    row += sbv[None, :]
    return np.ascontiguousarray(
        np.broadcast_to(row[:, None, :], (B, S, H)).astype(np.complex64))


# revision 24
# speedup vs baseline: 1.1695x; 1.1695x over previous
"""Trainium2 Bass kernel for nn_MultiHeadAttention_37538014167348.

The reference einsum is 'bhqk,bhvd->bhqd' (k and v are independent), so the
attention output factorizes into (sum_k softmax_weights) * (sum_v V). Softmax
rows sum to exactly 1 (also true for the complex softmax), hence:

    out[b, q, :] = (sum_s x[b, s, :]) @ Wv + S * bv     (independent of q)

Q/K/mask/softmax drop out entirely. The kernel computes the row-sum of x and a
complex [1,768]x[768,768] matvec; the host broadcasts the resulting row over
the 1024 sequence positions.

Sharding over 8 cores: (batch b in 0..3) x (contraction/feature half). Core
(b, j) reads x[b, :, j*384:(j+1)*384] (all 1024 rows, half the features,
3.15MB) and Wv[j*384:(j+1)*384, :] (half the weight rows, full 768 output
columns, 1.18MB bf16), and produces the partial matvec y_bj = u_bj @ Wv[half].
The host sums the two partials per batch and adds S*bv: no cross-core
communication, and per-core DMA drops from 7.44MB (previous version) to
4.33MB, which is what bounds the kernel (HBM ~358GB/s).

Pipeline (per core): x arrives as two column slabs: cols [0:512) as 8
row-subs of 128 rows (2KB DMA elements, on the two HWDGE queues), and cols
[512:768) as 4 pair-packed row-subs (1KB elements, on the gpsimd queue with
the weights). Stage 1 reduces rows with a stationary ones [128,2] f32r matmul
(free size 512 keeps the PE at 1 cycle/row) into psum uA [2,512] and uB
[2,512] (= even-row | odd-row partials, folded during the deinterleave adds).
Per 128-complex-feature chunk: DVE deinterleaves re/im psum columns into
rows, one PE transpose yields u columns [a|b], DVE packs bf16 [a,b] and
[-b,a] pairs, and 4 bf16 matmuls per chunk accumulate y into psum
[2,512]+[2,256] against the merged weight planes [C_cc | D_cc]. A dummy
matmul chain spans the DMA lead-in so the PE clock is ramped when real work
starts; constants are built on-chip (tiny-packet DMAs poison a queue).
Output is bf16 [2,768] (cast split across DVE and Act engines).
"""

import os
import sys

import numpy as np

for _p in ("/opt/trn_rl_repo", "/root/.axon_site/_ro/trn_rl_repo"):
    if os.path.isdir(_p) and _p not in sys.path:
        sys.path.append(_p)

import ml_dtypes

from concourse import bacc, mybir
from concourse.tile import TileContext
from concourse.bass_utils import run_bass_kernel_spmd

B, S, H = 4, 1024, 768
HALF = H // 2           # complex features per core (384) = contraction half
NCORES = 8
P = 128                 # SBUF partitions
CC = HALF // P          # 3 contraction chunks of 128 complex features
F32 = mybir.dt.float32
F32R = mybir.dt.float32r
BF16 = mybir.dt.bfloat16

_NC = None
LAST_RESULTS = None     # stashed BassKernelResults for profiling in test.py


def _build():
    nc = bacc.Bacc(None, target_bir_lowering=False)

    # per-core x half: f32 view of x[b, :, j*384:(j+1)*384], contiguous
    x = nc.dram_tensor("x", [S, 2 * HALF], F32R, kind="ExternalInput")
    # merged per-chunk weight planes [C_cc | D_cc]: w{cc}[p, 0:768] =
    # bf16(Re(Wv)[joff+cc*128+p, :]), [768:1536] = Im. 3KB rows -> best DMA
    # packet size.
    ws = [nc.dram_tensor(f"w{cc}", [P, 2 * H], BF16, kind="ExternalInput")
          for cc in range(CC)]
    idf = nc.dram_tensor("idf", [2, 512], F32, kind="ExternalInput")
    o = nc.dram_tensor("o", [2, H], BF16, kind="ExternalOutput")

    with TileContext(nc) as tc:
        with tc.tile_pool(name="sbuf", bufs=1) as pool, \
             tc.tile_pool(name="psum", bufs=1, space="PSUM") as psum:

            onesP = pool.tile([P, 2], F32R)
            ones_f = pool.tile([P, 2], F32)
            id2f = pool.tile([2, 512], F32)
            dummy_f = pool.tile([P, 512], F32)
            dummy_mv = pool.tile([P, 512], F32R)
            w_sb = [pool.tile([P, 2 * H], BF16, name=f"wsb{cc}") for cc in range(CC)]

            # slab01: x f32 cols [0:512) as 8 row-subs of 128 rows (2KB DMA
            # elements); group g holds rows 128g+p -> 8 free-512 matmuls.
            t01 = pool.tile([P, 8, 512], F32R)
            # slab2: x f32 cols [512:768) as 4 row-subs of 256 rows (1KB DMA
            # elements); partition p of sub s holds rows 256s+2p (group 2s)
            # and 256s+2p+1 (group 2s+1): one free-512 matmul per sub into
            # uB [2,512] = [even-row feats | odd-row feats], folded during
            # the deinterleave adds.
            t2 = pool.tile([P, 8, 256], F32R)

            # ---- on-chip constants: NEVER DMA [128, small] tiles (tiny
            # packets poison the queue). ones and the transpose identity are
            # built with memsets; id2's diagonal writes use partition-offset
            # memsets.
            nc.gpsimd.memset(ones_f[:], 1.0)
            nc.gpsimd.memset(dummy_f[:], 0.0)
            nc.vector.tensor_copy(onesP[:], ones_f[:])
            nc.vector.tensor_copy(dummy_mv[:], dummy_f[:])
            # identity rides as [2, 512] f32 = two 2KB packets (tiny-packet
            # transfers poison a queue); only [0:2, 0:2] is used.
            nc.gpsimd.dma_start(out=id2f[:], in_=idf[:, :])
            id2 = id2f[0:2, 0:2]

            # ---- DMA triggers. Queues sustain ~118GB/s each (aggregate
            # HBM-bound ~350GB/s); scalar/sync HWDGE start ~8.8us, gpsimd
            # swdge ~10.5us. The a-slab (stage-1 critical) goes first
            # everywhere; weights mid-stream; the last bytes are z2/w2 whose
            # dependent chains are shortest.
            def asub(g, eng):
                eng.dma_start(out=t01[:, g, :],
                              in_=x[P * g:P * (g + 1), 0:512])

            def zsub(s, eng):
                eng.dma_start(out=t2[:, 2 * s:2 * s + 2, :],
                              in_=x[256 * s:256 * (s + 1), 512:768])

            asub(0, nc.scalar)
            asub(1, nc.sync)
            asub(2, nc.gpsimd)
            asub(3, nc.scalar)
            asub(4, nc.sync)
            asub(5, nc.gpsimd)
            asub(6, nc.scalar)
            asub(7, nc.sync)
            nc.scalar.dma_start(out=w_sb[0][:], in_=ws[0][:, :])
            nc.sync.dma_start(out=w_sb[1][:], in_=ws[1][:, :])
            zsub(0, nc.gpsimd)
            zsub(1, nc.scalar)
            zsub(2, nc.sync)
            zsub(3, nc.gpsimd)
            nc.gpsimd.dma_start(out=w_sb[2][:], in_=ws[2][:, :])

            # ---- psum: uA (feats 0-511), uB (feats 512-767 pair-folded),
            # transposes, stage-2 accumulators, and the warm-up scratch.
            uA = psum.tile([2, 512], F32)
            uB = psum.tile([2, 512], F32)
            u_row = [pool.tile([2, P], F32, name=f"urow{cc}") for cc in range(CC)]
            tp = [psum.tile([P, 2], F32, name=f"tp{cc}") for cc in range(CC)]
            u_ab = [pool.tile([P, 2], BF16, name=f"uab{cc}") for cc in range(CC)]
            u_bna = [pool.tile([P, 2], BF16, name=f"ubna{cc}") for cc in range(CC)]
            oA = psum.tile([2, 512], F32)
            oB = psum.tile([2, 256], F32)
            ub_sb = pool.tile([2, 256], F32)
            scratch = psum.tile([2, 512], F32)

            def deint(cc):
                # u_row[cc]: row0 = a (Re), row1 = b (Im), feats cc*128..+127
                if cc < 2:
                    v = uA.rearrange("q (f two) -> q two f", two=2)
                    base = cc * P
                    nc.vector.tensor_copy(u_row[cc][0:2, :],
                                          v[0:2, 1, base:base + P])
                    nc.vector.tensor_copy(u_row[cc][0:1, :],
                                          v[0:1, 0, base:base + P])
                else:
                    # fold even-row + odd-row halves while deinterleaving;
                    # tensor_add reads at most one PSUM input, so bounce the
                    # even half through SBUF.
                    nc.vector.tensor_copy(ub_sb[:], uB[:, 0:256])
                    vp = uB.rearrange("q (g f two) -> q g two f", g=2, two=2)
                    vs = ub_sb.rearrange("q (f two) -> q two f", two=2)
                    nc.vector.tensor_add(u_row[cc][0:2, :],
                                         vp[0:2, 1, 1, :], vs[0:2, 1, :])
                    nc.vector.tensor_add(u_row[cc][0:1, :],
                                         vp[0:1, 1, 0, :], vs[0:1, 0, :])

            def pack(cc):
                nc.tensor.transpose(tp[cc][:, :], u_row[cc][0:2, :], id2)
                nc.vector.tensor_copy(u_ab[cc][:], tp[cc][:])
                nc.vector.tensor_scalar_mul(u_bna[cc][:, 0:1], tp[cc][:, 1:2], -1.0)
                nc.vector.tensor_copy(u_bna[cc][:, 1:2], tp[cc][:, 0:1])

            def stage2(cc, first, last):
                if not last:
                    nc.tensor.matmul(oA[:, :], u_ab[cc][:, :],
                                     w_sb[cc][:, 0:512], start=first, stop=False)
                    nc.tensor.matmul(oB[:, :], u_ab[cc][:, :],
                                     w_sb[cc][:, 512:768], start=first, stop=False)
                    nc.tensor.matmul(oA[:, :], u_bna[cc][:, :],
                                     w_sb[cc][:, 768:1280], start=False, stop=False)
                    nc.tensor.matmul(oB[:, :], u_bna[cc][:, :],
                                     w_sb[cc][:, 1280:1536], start=False, stop=False)
                else:
                    # close oA first: its bf16 cast (DVE, ~0.7us on only 2
                    # partitions) overlaps the remaining oB matmuls
                    nc.tensor.matmul(oA[:, :], u_ab[cc][:, :],
                                     w_sb[cc][:, 0:512], start=False, stop=False)
                    nc.tensor.matmul(oA[:, :], u_bna[cc][:, :],
                                     w_sb[cc][:, 768:1280], start=False, stop=True)
                    nc.tensor.matmul(oB[:, :], u_ab[cc][:, :],
                                     w_sb[cc][:, 512:768], start=False, stop=False)
                    nc.tensor.matmul(oB[:, :], u_bna[cc][:, :],
                                     w_sb[cc][:, 1280:1536], start=False, stop=True)

            # ---- PE warm-up: the clock ramps only under ~3us of continuous
            # execution; idle resets it. Wide dummy matmuls span the DMA
            # lead-in so the real chain starts warm.
            NWARM = 7
            for i in range(NWARM):
                nc.tensor.matmul(scratch[:, :], onesP[:, 0:2], dummy_mv[:, :],
                                 start=(i == 0), stop=(i == NWARM - 1))

            def dummy(n):
                # single-shot scratch matmuls: keep the PE clock ramped
                # through known DMA-stall windows (idle resets the ramp and
                # drops subsequent matmuls to ~1.6x cycle time)
                for _ in range(n):
                    nc.tensor.matmul(scratch[:, :], onesP[:, 0:2],
                                     dummy_mv[:, :], start=True, stop=True)

            # accumulation order is free: gpsimd's subs (a2, a5) arrive
            # latest, so they close the chain instead of head-of-line
            # blocking matmuls whose data is already resident.
            ua_order = (0, 1, 3, 4, 6, 7, 2, 5)
            for i, g in enumerate(ua_order):
                nc.tensor.matmul(uA[:, :], onesP[:, 0:2], t01[:, g, :],
                                 start=(i == 0), stop=(i == 7))
                if g == 2:
                    dummy(3)
            dummy(1)
            deint(0)
            pack(0)
            deint(1)
            pack(1)
            # uB in expected arrival order (z0 gpsimd, z1 scalar, z2 sync,
            # z3 gpsimd)
            for i, s in enumerate(range(4)):
                nc.tensor.matmul(uB[:, :], onesP[:, 0:2],
                                 t2[:, 2 * s:2 * s + 2, :],
                                 start=(i == 0), stop=(i == 3))
            deint(2)
            pack(2)
            stage2(0, True, False)
            stage2(1, False, False)
            stage2(2, False, True)

            o_sb = pool.tile([2, H], BF16)
            nc.vector.tensor_copy(o_sb[:, 0:512], oA[:])
            nc.scalar.mul(o_sb[:, 512:768], oB[:], 1.0)
            nc.sync.dma_start(out=o[:, :], in_=o_sb[:])

    nc.finalize()
    return nc


def _get_nc():
    global _NC
    if _NC is None:
        _NC = _build()
    return _NC


def _pack_w(Wv, j):
    # merged per-chunk planes [128, 2H]: cols 0:768 = Re rows, 768: = Im rows
    out = []
    for cc in range(CC):
        rows = slice(j * HALF + cc * P, j * HALF + (cc + 1) * P)
        wq = np.empty((P, 2 * H), dtype=ml_dtypes.bfloat16)
        wq[:, 0:H] = Wv.real[rows, :].astype(ml_dtypes.bfloat16)
        wq[:, H:2 * H] = Wv.imag[rows, :].astype(ml_dtypes.bfloat16)
        out.append(np.ascontiguousarray(wq))
    return out


def make_in_maps(x, Wv, bv):
    xf = np.ascontiguousarray(x).view(np.float32).reshape(B, S, 2 * H)
    Wv = np.ascontiguousarray(Wv)
    idv = np.zeros((2, 512), dtype=np.float32)
    idv[0, 0] = 1.0
    idv[1, 1] = 1.0
    wmaps = [_pack_w(Wv, j) for j in range(2)]
    in_maps = []
    for core in range(NCORES):
        b, j = divmod(core, 2)
        xc = np.ascontiguousarray(xf[b][:, j * 2 * HALF:(j + 1) * 2 * HALF])
        im = {"x": xc, "idf": idv}
        for cc in range(CC):
            im[f"w{cc}"] = wmaps[j][cc]
        in_maps.append(im)
    return in_maps


def kernel(x, Wq, bq, Wk, bk, Wv, bv, mask, trace=False):
    global LAST_RESULTS
    x = np.asarray(x)
    Wv = np.asarray(Wv)
    bv = np.asarray(bv)
    in_maps = make_in_maps(x, Wv, bv)
    res = run_bass_kernel_spmd(_get_nc(), in_maps, core_ids=list(range(NCORES)),
                               trace=trace)
    LAST_RESULTS = res
    sbv = (np.complex64(S) * bv).astype(np.complex64)
    row = np.empty((B, H), dtype=np.complex64)
    for b in range(B):
        o0 = res.results[2 * b]["o"].astype(np.float32)      # [2,768] bf16
        o1 = res.results[2 * b + 1]["o"].astype(np.float32)
        row[b] = (o0[0] + o1[0]) + 1j * (o0[1] + o1[1])
    row += sbv[None, :]
    return np.ascontiguousarray(
        np.broadcast_to(row[:, None, :], (B, S, H)).astype(np.complex64))
